# revision 18
# baseline (speedup 1.0000x reference)
"""Trainium2 Bass kernel for nn_ConvAttention_34600256537137.

Math notes (validated against the reference):
  qkv = 1x1conv(x, w1)+b1 -> Q,K,V;  score = conv5x5(Q_s)+conv5x5(K_t)+b2;
  attn = softmax_t(score);  out = einsum(attn, V).
  Softmax over t is shift-invariant, so the Q-half of the score (constant in
  t), b2, and the K-path bias all cancel.  The computation collapses to:
    weff[ci,dy,dx] = sum_c w1K[c,ci] * w2K[c,dy,dx]        (host, tiny)
    sK[b,t,h,w]    = conv5x5_reflect(x[b,:,:,:,t], weff)
    e = exp(sK);  den = sum_t e
    out[b,o,h,w,s] = (sum_{ci,t} w1V[o,ci] * e * x) / den + b1V[o]
  (s-independent; normalization folded to the end; bias added on host)

Sharding: 8 cores = (b in {0,1}) x (4 chunks of 8 rows of H).

v5 structure (bf16 matmul paths, tolerance is 2e-2):
  - conv5x5 via row-parity packing: contract dim = (ci, row%2) = 128, the
    stationary [128, 50] holds weff with a parity mask so each of the 12
    matmul chunks (row-pair, w-half) computes T'[(dy,j',dx), t, rp, w'] =
    per-tap partial conv sums; streams each slab position exactly once.
  - tap reduce: bounce T' to DRAM contiguously (5 fat descriptors), then 10
    gathers whose contiguous 144-elem runs span (a, w) for one (t, dx) --
    the rp shift of each (dy, j') is absorbed into the descriptor base.
    One 5-dim strided XY tensor_reduce sums all 25 taps.
  - output rows are permuted h = 2a+j' (h' = j'*4+a) host-side so gather
    destinations are contiguous partition blocks; positions live on 32
    partitions (j', t) with (a, w) in the free dim until the e broadcast.
  - e broadcast via one [t,(j',a,w)]-ordered DRAM bounce + one stride-0
    gather; 1/den broadcast via stride-0 gather.
  - V path: xattn = x*e in bf16, 8 accumulating bf16 matmuls, normalize on
    the PSUM read; output stored bf16, upcast + bias on host.
"""

import sys

if "/opt/trn_rl_repo" not in sys.path:
    sys.path.insert(0, "/opt/trn_rl_repo")

import numpy as np
import ml_dtypes

BF16 = ml_dtypes.bfloat16

B, C, H, W, S = 2, 64, 32, 32, 16
KS, PAD = 5, 2
NCORES = 8
ROWS = H // 4            # output rows per core (8)
SLAB_R = ROWS + 2 * PAD  # 12
RP = SLAB_R // 2         # 6 row pairs
SLAB_W = W + 2 * PAD     # 36
NSTA = 2 * KS * KS       # 50 stationary cols: (dy, j', dx)
HW = ROWS * W            # 256 output positions
FP = S * RP * SLAB_W     # 3456 free positions per T' partition
RUN = 4 * SLAB_W         # 144: contiguous (a, w') gather run

_MODULE = None


def _build_module():
    import concourse.bacc as bacc
    import concourse.bass as bass
    import concourse.tile as tile
    from concourse import mybir

    f32 = mybir.dt.float32
    bf16 = mybir.dt.bfloat16
    AF = mybir.ActivationFunctionType
    ALU = mybir.AluOpType
    nc = bacc.Bacc("TRN2", target_bir_lowering=False, debug=False, num_devices=NCORES)

    slab_d = nc.dram_tensor("slab", [128, RP, SLAB_W, S], bf16, kind="ExternalInput")
    sta_d = nc.dram_tensor("sta", [128, NSTA], bf16, kind="ExternalInput")
    xt_d = nc.dram_tensor("xt", [128, 8, HW], bf16, kind="ExternalInput")
    w1vr_d = nc.dram_tensor("w1vr", [128, 8, C], bf16, kind="ExternalInput")
    hsel_d = nc.dram_tensor("hsel", [128, ROWS], bf16, kind="ExternalInput")
    o_d = nc.dram_tensor("o", [C, S, HW], bf16, kind="ExternalOutput")

    # scratch DRAM for partition-crossing rearrangements
    td_d = nc.dram_tensor("td", [NSTA * FP], bf16)         # T' flat, same layout
    ed_d = nc.dram_tensor("ed", [S, HW], bf16)             # [t, (j', a, w)]
    dend_d = nc.dram_tensor("dend", [HW], f32)             # 1/den, flat (j', a, w)

    rot = [0]

    def dma(out, in_):
        e = (nc.sync, nc.scalar, nc.gpsimd)[rot[0] % 3]
        rot[0] += 1
        e.dma_start(out, in_)

    with tile.TileContext(nc) as tc:
        with tc.tile_pool(name="sb", bufs=1) as sb, tc.tile_pool(
            name="ps", bufs=6, space="PSUM"
        ) as ps, tc.tile_pool(name="pso", bufs=1, space="PSUM") as pso:
            # --- loads: slab in 6 single-rp chunks so the first conv matmul
            # waits on a 147KB transfer, not 294KB ---
            s_sta = sb.tile([128, NSTA], bf16)
            nc.scalar.dma_start(s_sta, sta_d.ap())
            s_hsel = sb.tile([128, ROWS], bf16)
            nc.gpsimd.dma_start(s_hsel, hsel_d.ap())
            slab_t = []
            slab_eng = (nc.sync, nc.scalar, nc.gpsimd)
            for rp in range(RP):
                st = sb.tile([128, SLAB_W, S], bf16, tag=f"slab{rp}")
                slab_t.append(st)
                slab_eng[rp % 3].dma_start(st, slab_d.ap()[:, rp, :, :])
            s_xt = sb.tile([128, 8, HW], bf16)
            nc.sync.dma_start(s_xt, xt_d.ap())
            s_w1vr = sb.tile([128, 8, C], bf16)
            nc.scalar.dma_start(s_w1vr, w1vr_d.ap())

            # --- phase 1: T'[(dy,j',dx), t, rp, w'] = sta^T @ slab chunks ---
            s_T2 = sb.tile([NSTA, S, RP, SLAB_W], bf16)
            HREST = SLAB_W // 2  # 18
            for ch in range(12):
                rp, half = divmod(ch, 2)
                p_t = ps.tile([NSTA, HREST, S], f32, tag="pt")
                nc.tensor.matmul(
                    p_t,
                    s_sta,
                    slab_t[rp][:, half * HREST : (half + 1) * HREST, :],
                    start=True,
                    stop=True,
                )
                eng = nc.vector if ch % 2 == 0 else nc.scalar
                if eng is nc.vector:
                    eng.tensor_copy(
                        s_T2[:, :, rp, half * HREST : (half + 1) * HREST],
                        p_t.transpose([0, 2, 1]),
                    )
                else:
                    eng.copy(
                        s_T2[:, :, rp, half * HREST : (half + 1) * HREST],
                        p_t.transpose([0, 2, 1]),
                    )

            # --- bounce T' to DRAM per (dy,j') keeping only the 4 needed rp
            # rows (a = rp - s, s=(dy+j')//2) as [dx, t, a, w'] blocks, then
            # gather onto 128 partitions (j',t,a) with (t,a) merged on the
            # source side; w-runs of 36 cover all 5 dx shifts ---
            BLK = 5 * S * 4 * SLAB_W  # 11520 elements per (dy,j') block
            s_R = sb.tile([128, 5, 5, SLAB_W], bf16)
            # all bounces first (in-order queues: a gather's sem wait would
            # stall every later descriptor on its engine), natural (dy,jp)
            # order is ascending in s so early bounces need only early copies
            brot = [0]
            for dy in range(5):
                for jp in range(2):
                    s0 = (dy + jp) // 2
                    bd = (dy * 2 + jp) * BLK
                    p0 = (dy * 2 + jp) * 5
                    dst = bass.AP(
                        tensor=td_d.ap().tensor,
                        offset=bd,
                        ap=[[4 * SLAB_W * S, 5], [4 * SLAB_W, S], [1, 4 * SLAB_W]],
                    )
                    (nc.sync, nc.gpsimd)[brot[0] % 2].dma_start(
                        dst, s_T2[p0 : p0 + 5, :, s0 : s0 + 4, :]
                    )
                    brot[0] += 1
            for dy in range(5):
                for jp in range(2):
                    bd = (dy * 2 + jp) * BLK
                    src = bass.AP(
                        tensor=td_d.ap().tensor,
                        offset=bd,
                        ap=[[SLAB_W, 64], [4 * SLAB_W * S, 5], [1, SLAB_W]],
                    )
                    dma(s_R[jp * 64 : (jp + 1) * 64, dy, :, :], src)

            # --- tap reduce: tap (dy,dx) value for output w sits at free
            # offset dy*180 + dx*37 + w; one 4-dim XY reduce on 128 lanes ---
            s_sk = sb.tile([128, W], f32)
            view = bass.AP(
                tensor=s_R.tensor,
                offset=s_R.offset,
                ap=[[s_R.ap[0][0], 128], [1, W], [5 * SLAB_W, 5], [SLAB_W + 1, 5]],
            )
            nc.vector.tensor_reduce(
                s_sk, view, axis=mybir.AxisListType.XY, op=ALU.add
            )

            # --- e = exp(sK) ---
            s_e = sb.tile([128, W], bf16)
            nc.scalar.activation(s_e, s_sk, AF.Exp)

            # --- e broadcast first (critical path): [t,(j',a,w)]-ordered
            # bounce (2 desc) + gather in halves on separate queues ---
            for jp in range(2):
                ed_dst = bass.AP(
                    tensor=ed_d.ap().tensor,
                    offset=jp * 4 * W,
                    ap=[[HW, S], [1, 4 * W]],
                )
                (nc.sync, nc.gpsimd)[jp].dma_start(
                    ed_dst, s_e[jp * 64 : (jp + 1) * 64, :]
                )
            s_eb = sb.tile([128, HW], bf16)
            for g2 in range(2):
                (nc.sync, nc.gpsimd)[g2].dma_start(
                    s_eb[g2 * 64 : (g2 + 1) * 64, :],
                    bass.AP(
                        tensor=ed_d.ap().tensor,
                        offset=0,
                        ap=[[0, 4], [HW, S], [1, HW]],
                    ),
                )

            # --- den via indicator-matmul on PE (off critical path) ---
            p_den = pso.tile([ROWS, W], f32, tag="den")
            nc.tensor.matmul(p_den, s_hsel, s_e, start=True, stop=True)
            s_rcp = sb.tile([ROWS, W], f32)
            nc.vector.reciprocal(s_rcp, p_den)
            nc.scalar.dma_start(dend_d.ap(), s_rcp)
            s_rcpb = sb.tile([C, HW], f32)
            nc.scalar.dma_start(
                s_rcpb,
                bass.AP(tensor=dend_d.ap().tensor, offset=0, ap=[[0, C], [1, HW]]),
            )

            # --- V path: xattn = x_t * e; contract (ci,t) on PE ---
            s_xa = sb.tile([128, 8, HW], bf16)
            nc.vector.tensor_tensor(
                s_xa,
                s_xt,
                s_eb.unsqueeze(1).broadcast_to((128, 8, HW)),
                op=ALU.mult,
            )
            p_o = pso.tile([C, HW], f32, tag="out")
            for g in range(8):
                nc.tensor.matmul(
                    p_o,
                    s_w1vr[:, g, :],
                    s_xa[:, g, :],
                    start=(g == 0),
                    stop=(g == 7),
                )
            # normalize on the PSUM->SBUF read
            s_o = sb.tile([C, HW], bf16)
            nc.vector.tensor_tensor(s_o, p_o, s_rcpb, op=ALU.mult)
            bounds = [0, 22, 43, C]
            for ci, e in enumerate((nc.sync, nc.scalar, nc.gpsimd)):
                a, b = bounds[ci], bounds[ci + 1]
                e.dma_start(
                    o_d.ap()[a:b],
                    s_o[a:b].unsqueeze(1).broadcast_to((b - a, S, HW)),
                )

    nc.compile()
    return nc


def _get_module():
    global _MODULE
    if _MODULE is None:
        _MODULE = _build_module()
    return _MODULE


# h' = j'*4 + a  <->  h = 2a + j'
HMAP = [2 * (i % 4) + (i // 4) for i in range(8)]  # h of h'


def make_host_inputs(x, w1, b1, w2, b2):
    """Host-side precompute: folded weights + per-core reflect-padded slices."""
    x = np.ascontiguousarray(np.asarray(x, np.float32))
    w1 = np.asarray(w1, np.float32)
    w2 = np.asarray(w2, np.float32)

    w1K = w1[C : 2 * C, :, 0, 0]          # [c, ci]
    w2K = w2[0, C : 2 * C]                # [c, 5, 5]
    weff = np.einsum("ci,cyx->iyx", w1K, w2K)  # [ci, dy, dx]
    w1V = w1[2 * C :, :, 0, 0]            # [co, ci]

    # sta[(ci,j), (dy,j',dx)] = weff[ci,dy,dx] * [j == (dy+j') % 2]
    sta = np.zeros((C, 2, 5, 2, 5), np.float32)
    for dy in range(5):
        for jp in range(2):
            sta[:, (dy + jp) % 2, dy, jp, :] = weff[:, dy, :]
    sta = np.ascontiguousarray(sta.reshape(128, NSTA).astype(BF16))

    # w1vr[(ci8,t), g, co] = w1V[co, 8g+ci8]
    tmp = w1V.T.reshape(8, 8, C)                      # (g, ci8, co)
    w1vr = np.ascontiguousarray(
        np.broadcast_to(tmp[:, :, None, :], (8, 8, S, C))
        .transpose(1, 2, 0, 3)
        .reshape(128, 8, C)
        .astype(BF16)
    )

    # hsel[(j',t,a), m] = 1 iff m == h' = j'*4 + a
    hsel = np.zeros((128, ROWS), np.float32)
    for p in range(128):
        jp, a = p // 64, p % 4
        hsel[p, jp * 4 + a] = 1.0
    hsel = hsel.astype(BF16)

    in_maps = []
    for core in range(NCORES):
        b, hc = divmod(core, 4)
        h0 = ROWS * hc
        xp = np.pad(x[b], ((0, 0), (PAD, PAD), (PAD, PAD), (0, 0)), mode="reflect")
        sl = xp[:, h0 : h0 + SLAB_R, :, :]            # [ci, row, w', t]
        slab = np.ascontiguousarray(
            sl.reshape(C, RP, 2, SLAB_W, S)
            .transpose(0, 2, 1, 3, 4)
            .reshape(128, RP, SLAB_W, S)
            .astype(BF16)
        )
        xs = x[b][:, h0 : h0 + ROWS][:, HMAP]          # [ci, h'(8), w, t]
        xt = np.ascontiguousarray(
            xs.reshape(8, 8, ROWS, W, S)
            .transpose(1, 4, 0, 2, 3)
            .reshape(128, 8, HW)
            .astype(BF16)
        )
        in_maps.append(
            {"slab": slab, "xt": xt, "sta": sta, "w1vr": w1vr, "hsel": hsel}
        )
    return in_maps


def assemble_output(results, b1):
    b1V = np.asarray(b1, np.float32)[2 * C :]
    out = np.empty((B, C, H, W, S), np.float32)
    for core in range(NCORES):
        b, hc = divmod(core, 4)
        h0 = ROWS * hc
        o = np.asarray(results[core]["o"], np.float32).reshape(C, S, ROWS, W)
        for i in range(ROWS):
            out[b, :, h0 + HMAP[i], :, :] = o[:, :, i, :].transpose(0, 2, 1)
    out += b1V[None, :, None, None, None]
    return out


def kernel(x, w1, b1, w2, b2):
    from concourse.bass_utils import run_bass_kernel_spmd

    nc = _get_module()
    in_maps = make_host_inputs(x, w1, b1, w2, b2)
    res = run_bass_kernel_spmd(nc, in_maps, core_ids=list(range(NCORES)))
    return assemble_output(res.results, b1)
